# revision 1
# baseline (speedup 1.0000x reference)
"""Chamfer-loss-overlap kernel for 8 Trainium2 NeuronCores.

Math (per batch element, reference semantics):
    P[i,j] = |x_i|^2 + |y_j|^2 - 2 x_i . y_j          (4096 x 4096)
    a = mean(x_mask * min_i P[i,j])    (min over i, per y-point j)
    b = mean(y_mask * min_j P[i,j])    (min over j, per x-point i)
    out = (a - b)^2

Sharding: batch dim B=8 across the 8 cores (data parallel). Each core
computes its 4096x4096 distance matrix ONCE and extracts both min
directions from the same PSUM tiles:
  - row-min (min over j per x-point i): free-dim reduce of the bf16 copy
  - col-min (min over i per y-point j): running elementwise bf16 min
    across the 32 row-tiles, then a TensorE-transpose + reduce tail to
    fold the remaining 128 partitions.
Host applies masks / means in float64 and squares the difference.

Device kernel strategy:
  - The distance matrix is produced on TensorE as ONE K=13 bf16 matmul per
    128x512 tile: fp32 x/y are split hi/lo into bf16 (x ~ xh + xl), and the
    |x|^2 / |y|^2 terms ride along as extra contraction rows against ones.
  - ScalarE casts each PSUM strip to bf16 in SBUF; VectorE does the
    row-min (fold + reduce); the col-min accumulation (tensor_tensor min
    at the DVE 2x bf16 rate) runs on VectorE and/or GpSimd.
  - bf16 rounding is monotone, so min(bf16(P)) == bf16(min(P)): both
    reductions see identically-rounded values.
"""

import numpy as np
from ml_dtypes import bfloat16

import concourse.bacc as bacc
import concourse.bass as bass
import concourse.mybir as mybir
from concourse import tile
from concourse import masks

B, N, D = 8, 4096, 3
NCORES = 8
NT = N // 128        # 32 row-tiles
QW = 512             # one PSUM bank of fp32 (max matmul free dim)
K = 13               # contraction rows of the augmented matmul

SW = 1024            # PSUM strip width (2 banks)
NSTRIP = N // SW     # 4 strips per row-tile
PSUM_BUFS = 4        # 4 x 2 banks = all 8 banks
CPY_BUFS = 3         # [128, N] bf16 staging buffers

# --- tuning config ----------------------------------------------------
# nbands: 1 = plain 128x128 PE, 4 = pack K=13 matmuls into the 4 32-row
#         PE bands (tile_position) so they stream concurrently.
# row_mode: "fold" = 3x tensor_tensor min then reduce on [128,1024]
#           "ttr"  = one fused tensor_tensor_reduce over the two halves
#           "direct" = single tensor_reduce over [128,4096]
# gp_cols: number of colmin columns handled by GpSimd (rest on VectorE)
# sw: PSUM strip width (psum_bufs * sw * 4B must fit in 16KB/partition)
DEFAULT_CFG = dict(
    nbands=1,
    row_mode="fold",
    gp_cols=0,
    sw=1024,
    psum_bufs=4,
    cpy_bufs=3,
    ablate=None,   # None | "row" | "col" (timing experiments only)
)
# ---------------------------------------------------------------------

_CACHE = {}


def _build_nc(reps=1, **overrides):
    cfg = dict(DEFAULT_CFG, **overrides)
    dt = mybir.dt
    amin = mybir.AluOpType.min
    nc = bacc.Bacc("TRN2", target_bir_lowering=False, debug=False,
                   num_devices=NCORES)

    la_d = nc.dram_tensor("la", [K, N], dt.bfloat16, kind="ExternalInput")
    ra_d = nc.dram_tensor("ra", [K, N], dt.bfloat16, kind="ExternalInput")
    minsA_d = nc.dram_tensor("minsA", [128, NT], dt.float32,
                             kind="ExternalOutput")
    minsB_d = nc.dram_tensor("minsB", [128, NT], dt.float32,
                             kind="ExternalOutput")

    with tile.TileContext(nc) as tc:
        with (
            tc.tile_pool(name="rows", bufs=1) as rows,
            tc.tile_pool(name="accs", bufs=1) as accs,
        ):
            npart = 96 + K if cfg["nbands"] == 4 else K
            la = rows.tile([npart, N], dt.bfloat16, tag="la")
            ra = rows.tile([npart, N], dt.bfloat16, tag="ra")
            for t, d in ((la, la_d), (ra, ra_d)):
                if cfg["nbands"] == 4:
                    for r in range(4):
                        nc.sync.dma_start(t[32 * r:32 * r + K, :], d[:])
                else:
                    nc.sync.dma_start(t[:], d[:])

            ident = rows.tile([128, 128], dt.bfloat16, tag="ident")
            masks.make_identity(nc, ident[:])

            colmin = accs.tile([128, N], dt.bfloat16, tag="colmin")
            rowacc = accs.tile([128, NT], dt.float32, tag="rowacc")

            import contextlib
            rep_ctx = (tc.For_i(0, reps, 1) if reps > 1
                       else contextlib.nullcontext())
            with rep_ctx:
                _emit_main(nc, tc, la, ra, colmin, rowacc, cfg)

            # tail: fold colmin's 128 partitions per column via TensorE
            # transpose (bf16, 128x128 blocks) + free-dim reduce from PSUM
            minsB_sb = accs.tile([128, NT], dt.float32, tag="minsB_sb")
            with tc.tile_pool(name="tpsum", bufs=2,
                              space=bass.MemorySpace.PSUM) as tpsum:
                for g in range(NT // 4):
                    pst = tpsum.tile([128, 4, 128], dt.bfloat16, tag="pst")
                    for b4 in range(4):
                        t = g * 4 + b4
                        nc.tensor.transpose(
                            pst[:, b4, :],
                            colmin[:, t * 128:(t + 1) * 128],
                            ident[:],
                        )
                    nc.vector.tensor_reduce(minsB_sb[:, g * 4:(g + 1) * 4],
                                            pst[:],
                                            axis=mybir.AxisListType.X,
                                            op=amin)
            nc.sync.dma_start(minsA_d[:], rowacc[:])
            nc.sync.dma_start(minsB_d[:], minsB_sb[:])

    nc.compile()
    return nc


def _emit_main(nc, tc, la, ra, colmin, rowacc, cfg):
    dt = mybir.dt
    amin = mybir.AluOpType.min
    sw = cfg["sw"]
    nstrip = N // sw
    nbands = cfg["nbands"]
    gp_cols = cfg["gp_cols"]
    with (
        tc.tile_pool(name="psum", bufs=cfg["psum_bufs"],
                     space=bass.MemorySpace.PSUM) as psum,
        tc.tile_pool(name="cpy", bufs=cfg["cpy_bufs"]) as cpy,
        tc.tile_pool(name="tmps", bufs=6) as tmps,
    ):
        for it in range(NT):
            i0 = it * 128
            cp = cpy.tile([128, N], dt.bfloat16, tag="cp", name="cp")
            for h in range(nstrip):
                bo = 32 * (h % nbands)
                ps = psum.tile([128, sw], dt.float32, tag="ps", name="ps")
                for q in range(sw // QW):
                    j0 = h * sw + q * QW
                    nc.tensor.matmul(
                        ps[:, q * QW:(q + 1) * QW],
                        la[bo:bo + K, i0:i0 + 128],
                        ra[bo:bo + K, j0:j0 + QW],
                        start=True, stop=True,
                        tile_position=(bo, 0) if nbands > 1 else None,
                    )
                nc.scalar.copy(cp[:, h * sw:(h + 1) * sw], ps[:])

            # running col-min (elementwise, bf16 2x), optionally split
            # between GpSimd (first gp_cols columns) and VectorE (rest)
            engs = []
            if gp_cols > 0:
                engs.append((nc.gpsimd, 0, gp_cols))
            engs.append((nc.vector, gp_cols, N))
            if cfg["ablate"] == "col" and it > 0:
                engs = []
            for eng, c0, c1 in engs:
                if c0 == c1:
                    continue
                sl = slice(c0, c1)
                if it == 0:
                    eng.tensor_copy(colmin[:, sl], cp[:, sl])
                else:
                    eng.tensor_tensor(colmin[:, sl], colmin[:, sl],
                                      cp[:, sl], amin)

            # row-min (free-dim)
            if cfg["ablate"] == "row":
                pass
            elif cfg["row_mode"] == "tsacc":
                # single 4x-capable single-src op: out(ignored) = cp * 1,
                # accum_out = reduce_min(out) -> the whole row-min in one go
                waste = tmps.tile([128, N], dt.bfloat16, tag="waste",
                                  name="waste")
                nc.vector.tensor_scalar(
                    waste[:], cp[:], 1.0, None,
                    op0=mybir.AluOpType.mult, op1=amin,
                    accum_out=rowacc[:, it:it + 1])
            elif cfg["row_mode"] == "fold2":
                # halving TT folds (2x rate) down to 256, then a short
                # 1x-rate reduce: minimizes time on the 1x reduce path
                src = cp
                w = N
                while w > 256:
                    w //= 2
                    dstt = tmps.tile([128, w], dt.bfloat16,
                                     tag=f"f{w}", name=f"f{w}")
                    nc.vector.tensor_tensor(dstt[:], src[:, 0:w],
                                            src[:, w:2 * w], amin)
                    src = dstt
                nc.vector.tensor_reduce(rowacc[:, it:it + 1], src[:],
                                        axis=mybir.AxisListType.X, op=amin)
            elif cfg["row_mode"] == "direct":
                nc.vector.tensor_reduce(rowacc[:, it:it + 1], cp[:],
                                        axis=mybir.AxisListType.X, op=amin)
            elif cfg["row_mode"] == "ttr":
                th = tmps.tile([128, N // 2], dt.bfloat16, tag="th",
                               name="th")
                nc.vector.tensor_tensor_reduce(
                    out=th[:],
                    in0=cp[:, 0:N // 2],
                    in1=cp[:, N // 2:N],
                    scale=1.0,
                    scalar=float(np.finfo(np.float32).max),
                    op0=amin,
                    op1=amin,
                    accum_out=rowacc[:, it:it + 1],
                )
            else:
                nq = N // 4
                t1 = tmps.tile([128, nq], dt.bfloat16, tag="t1", name="t1")
                t2 = tmps.tile([128, nq], dt.bfloat16, tag="t2", name="t2")
                t3 = tmps.tile([128, nq], dt.bfloat16, tag="t3", name="t3")
                nc.vector.tensor_tensor(t1[:], cp[:, 0:nq],
                                        cp[:, nq:2 * nq], amin)
                nc.vector.tensor_tensor(t2[:], cp[:, 2 * nq:3 * nq],
                                        cp[:, 3 * nq:4 * nq], amin)
                nc.vector.tensor_tensor(t3[:], t1[:], t2[:], amin)
                nc.vector.tensor_reduce(rowacc[:, it:it + 1], t3[:],
                                        axis=mybir.AxisListType.X, op=amin)


def get_nc():
    if "nc" not in _CACHE:
        _CACHE["nc"] = _build_nc()
    return _CACHE["nc"]


def _make_runner(nc):
    """Build a cached jitted SPMD callable for `nc` (one NEFF on all 8
    cores, per-core inputs sharded along axis 0)."""
    import jax
    from jax.sharding import Mesh, PartitionSpec
    from jax.experimental.shard_map import shard_map
    from concourse.bass2jax import (
        _bass_exec_p,
        install_neuronx_cc_hook,
        partition_id_tensor,
    )

    install_neuronx_cc_hook()
    partition_name = (nc.partition_id_tensor.name
                      if nc.partition_id_tensor else None)

    in_names = []
    out_names = []
    out_avals = []
    out_shapes = []
    for alloc in nc.m.functions[0].allocations:
        if not isinstance(alloc, mybir.MemoryLocationSet):
            continue
        name = alloc.memorylocations[0].name
        if alloc.kind == "ExternalInput":
            if name != partition_name:
                in_names.append(name)
        elif alloc.kind == "ExternalOutput":
            shape = tuple(alloc.tensor_shape)
            dtype = mybir.dt.np(alloc.dtype)
            out_avals.append(jax.core.ShapedArray(shape, dtype))
            out_names.append(name)
            out_shapes.append((shape, dtype))
    n_params = len(in_names)
    n_outs = len(out_names)
    all_names = list(in_names) + list(out_names)
    if partition_name is not None:
        all_names.append(partition_name)
    donate = tuple(range(n_params, n_params + n_outs))

    def _body(*args):
        operands = list(args)
        if partition_name is not None:
            operands.append(partition_id_tensor())
        outs = _bass_exec_p.bind(
            *operands,
            out_avals=tuple(out_avals),
            in_names=tuple(all_names),
            out_names=tuple(out_names),
            lowering_input_output_aliases=(),
            sim_require_finite=True,
            sim_require_nnan=True,
            nc=nc,
        )
        return tuple(outs)

    devices = jax.devices()[:NCORES]
    mesh = Mesh(np.asarray(devices), ("core",))
    sharded = jax.jit(
        shard_map(_body, mesh=mesh,
                  in_specs=(PartitionSpec("core"),) * (n_params + n_outs),
                  out_specs=(PartitionSpec("core"),) * n_outs,
                  check_rep=False),
        donate_argnums=donate,
        keep_unused=True,
    )

    def prep(in_maps):
        concat_in = [
            np.concatenate([np.asarray(m[name]) for m in in_maps], axis=0)
            for name in in_names
        ]
        return concat_in

    def exec_prepped(concat_in):
        concat_zeros = [
            np.zeros((NCORES * s[0], *s[1:]), dt) for s, dt in out_shapes
        ]
        return sharded(*concat_in, *concat_zeros)

    def unpack(out_arrs):
        return [
            {
                name: np.asarray(out_arrs[i]).reshape(
                    NCORES, *out_shapes[i][0])[c]
                for i, name in enumerate(out_names)
            }
            for c in range(NCORES)
        ]

    def run(in_maps):
        return unpack(exec_prepped(prep(in_maps)))

    run.prep = prep
    run.exec_prepped = exec_prepped
    run.unpack = unpack
    run.mesh = mesh
    return run


def get_runner():
    if "run" not in _CACHE:
        _CACHE["run"] = _make_runner(get_nc())
    return _CACHE["run"]


def _f32(v):
    return np.asarray(v, dtype=np.float32)


def _bf(v):
    return np.asarray(v, dtype=np.float32).astype(bfloat16)


def build_rows(xc, yc):
    """Build the two [13, 4096] bf16 row tensors for one batch element.

    Contraction layout (k : L-row      * R-row):
      0-2 : -2*xh_d  * yh_d
      3-5 : -2*xl_d  * yh_d
      6-8 : -2*xh_d  * yl_d
      9   : sqx_h    * 1
      10  : sqx_l    * 1
      11  : 1        * sqy_h
      12  : 1        * sqy_l
    """
    def side(v):
        vh = _bf(v)
        vl = _bf(_f32(v) - _f32(vh))
        sq = (np.asarray(v, np.float64) ** 2).sum(-1)
        sqh = _bf(sq)
        sql = _bf(sq - np.float64(1.0) * _f32(sqh).astype(np.float64))
        m2h = _bf(-2.0 * _f32(vh))
        m2l = _bf(-2.0 * _f32(vl))
        return vh, vl, sqh, sql, m2h, m2l

    xh, xl, sqxh, sqxl, m2xh, m2xl = side(xc)
    yh, yl, sqyh, sqyl, m2yh, m2yl = side(yc)
    ones = np.ones((N,), dtype=bfloat16)

    la = np.stack([m2xh[:, 0], m2xh[:, 1], m2xh[:, 2],
                   m2xl[:, 0], m2xl[:, 1], m2xl[:, 2],
                   m2xh[:, 0], m2xh[:, 1], m2xh[:, 2],
                   sqxh, sqxl, ones, ones])
    ra = np.stack([yh[:, 0], yh[:, 1], yh[:, 2],
                   yh[:, 0], yh[:, 1], yh[:, 2],
                   yl[:, 0], yl[:, 1], yl[:, 2],
                   ones, ones, sqyh, sqyl])
    return {
        "la": np.ascontiguousarray(la),
        "ra": np.ascontiguousarray(ra),
    }


def _mins_to_vec(m):
    # m[p, it] is the min for point index it*128 + p
    return np.asarray(m, np.float64).T.reshape(N)


def kernel(x, y, x_mask, y_mask):
    x = np.asarray(x)
    y = np.asarray(y)
    in_maps = [build_rows(x[c], y[c]) for c in range(B)]
    res = get_runner()(in_maps)

    sa = 0.0
    sb = 0.0
    for c in range(B):
        minsA = _mins_to_vec(res[c]["minsA"])  # min over j, per x-point i
        minsB = _mins_to_vec(res[c]["minsB"])  # min over i, per y-point j
        sa += (np.asarray(x_mask[c], np.float64) * minsB).sum()
        sb += (np.asarray(y_mask[c], np.float64) * minsA).sum()
    a = sa / (B * N)
    b = sb / (B * N)
    return np.asarray((a - b) ** 2, dtype=np.float32)

